# revision 20
# baseline (speedup 1.0000x reference)
"""Trainium2 Bass kernel for nn_AttentionLayer_10995116278518.

Computes softmax(einsum('sbe,e->bs', embedded, attn[:300])
              + einsum('sbf,f->bs', lstm_outputs, attn[300:]), axis=1)
(the reference's mask is computed-but-discarded, so it is unused here).

Sharding: data-parallel over batch. Each of the 8 cores handles 8 of the
64 batch rows; no cross-device communication.

Per-core device kernel layout:
  rows = (s, b) pairs; tiles put 128 consecutive s on partitions for a
  fixed b, features on the free axis. A fused VectorE tensor_tensor_reduce
  (multiply by the partition-broadcast attn vector, then add-reduce along
  the free axis) produces one dot product per partition. The per-row dots
  are collected as columns of L[128s, 4k*8b]; four PE transposes turn that
  into logits [8b, 512s] in PSUM, where softmax is a pure free-axis op.
"""

import os
import sys

import numpy as np

try:
    import concourse.bass as bass
except ImportError:  # stand-alone grading dir: the runtime lives here
    sys.path.insert(0, "/opt/trn_rl_repo")
    import concourse.bass as bass

import concourse.bacc as bacc
import concourse.tile as tile
from concourse import mybir
from concourse.bass_utils import run_bass_kernel_spmd

SEQ = 512
BATCH = 64
EMB = 300
LSTM = 4096
N_CORES = 8
BLOC = BATCH // N_CORES  # 8 batch rows per core
P = 128
NSB = SEQ // P  # 4 s-blocks of 128

F32 = mybir.dt.float32


def _build() -> bass.Bass:
    nc = bacc.Bacc()
    # host passes shards transposed to b-major: [BLOC, SEQ, feat]
    emb = nc.declare_dram_parameter("embedded", [BLOC, SEQ, EMB], F32, isOutput=False)
    lstm = nc.declare_dram_parameter(
        "lstm_outputs", [BLOC, SEQ, LSTM], F32, isOutput=False
    )
    # attn pre-broadcast to 128 partitions on the host (one contiguous load)
    attn_bc = nc.declare_dram_parameter("attn_bc", [P, EMB + LSTM], F32, isOutput=False)
    ident = nc.declare_dram_parameter("ident", [P, P], F32, isOutput=False)
    out = nc.declare_dram_parameter("out", [BLOC, SEQ], F32, isOutput=True)

    with tile.TileContext(nc) as tc:
        with (
            tc.tile_pool(name="singles", bufs=1) as singles,
            tc.tile_pool(name="lstm_tiles", bufs=6) as lstm_pool,
            tc.tile_pool(name="emb_tiles", bufs=2) as emb_pool,
            tc.tile_pool(name="psum", bufs=1, space="PSUM") as psum_pool,
        ):
            # attn + ident on the gpsimd ring so the sync ring's first loads
            # are lstm tiles
            sb_attn = singles.tile([P, EMB + LSTM], F32)
            nc.gpsimd.dma_start(out=sb_attn, in_=attn_bc[:, :])
            attn_e = sb_attn[:, 0:EMB]
            attn_l = sb_attn[:, EMB : EMB + LSTM]
            sb_ident = singles.tile([P, P], F32)
            nc.gpsimd.dma_start(out=sb_ident, in_=ident[:, :])

            # throwaway elementwise output of the ACT accumulating copy
            scratch = singles.tile([P, LSTM], F32)
            # per-row dot products: column k*BLOC+b holds rows (s=128k.., b)
            L = singles.tile([P, NSB * BLOC], F32)
            tmpl = singles.tile([P, NSB * BLOC], F32)  # lstm partials
            tmpe = singles.tile([P, NSB * BLOC], F32)  # embedded partials

            for k in range(NSB):
                emb_t = emb_pool.tile([P, BLOC, EMB], F32)
                # [s, b, f] gather from b-major DRAM (1200B contiguous runs)
                nc.gpsimd.dma_start(
                    out=emb_t,
                    in_=emb[:, k * P : (k + 1) * P, :].rearrange("b s f -> s b f"),
                )
                for b in range(BLOC):
                    col = k * BLOC + b
                    lstm_t = lstm_pool.tile([P, LSTM], F32)
                    # fully contiguous 2 MB read; split across both DMA paths
                    dma_eng = nc.sync if b % 2 == 0 else nc.gpsimd
                    dma_eng.dma_start(
                        out=lstm_t, in_=lstm[b, k * P : (k + 1) * P, :]
                    )
                    # VectorE multiplies by attn in place; ScalarE reduces
                    nc.vector.tensor_mul(lstm_t, lstm_t, attn_l)
                    nc.scalar.activation(
                        out=scratch,
                        in_=lstm_t,
                        func=mybir.ActivationFunctionType.Copy,
                        accum_out=tmpl[:, col : col + 1],
                    )
                    nc.vector.tensor_mul(emb_t[:, b, :], emb_t[:, b, :], attn_e)
                    nc.scalar.activation(
                        out=scratch[:, 0:EMB],
                        in_=emb_t[:, b, :],
                        func=mybir.ActivationFunctionType.Copy,
                        accum_out=tmpe[:, col : col + 1],
                    )

            nc.vector.tensor_add(L, tmpl, tmpe)

            # logits [8b, 512s] in PSUM via four PE transposes of [128, 8]
            logits = psum_pool.tile([BLOC, SEQ], F32)
            for k in range(NSB):
                nc.tensor.transpose(
                    out=logits[:, k * P : (k + 1) * P],
                    in_=L[:, k * BLOC : (k + 1) * BLOC],
                    identity=sb_ident,
                )

            # softmax along s (free axis)
            m = singles.tile([BLOC, 1], F32)
            nm = singles.tile([BLOC, 1], F32)
            ssum = singles.tile([BLOC, 1], F32)
            rec = singles.tile([BLOC, 1], F32)
            expt = singles.tile([BLOC, SEQ], F32)
            res = singles.tile([BLOC, SEQ], F32)
            nc.vector.reduce_max(out=m, in_=logits, axis=mybir.AxisListType.X)
            nc.vector.tensor_scalar_mul(nm, m, -1.0)
            nc.scalar.activation(
                out=expt,
                in_=logits,
                func=mybir.ActivationFunctionType.Exp,
                bias=nm,
                scale=1.0,
                accum_out=ssum,
            )
            nc.vector.reciprocal(rec, ssum)
            nc.vector.tensor_scalar_mul(res, expt, rec)
            nc.sync.dma_start(out=out[:, :], in_=res)

    nc.compile()
    return nc


_NC_CACHE = None


def _get_nc() -> bass.Bass:
    global _NC_CACHE
    if _NC_CACHE is None:
        _NC_CACHE = _build()
    return _NC_CACHE


def _make_in_maps(embedded, lstm_outputs, attn):
    embedded = np.asarray(embedded, dtype=np.float32)
    lstm_outputs = np.asarray(lstm_outputs, dtype=np.float32)
    attn = np.asarray(attn, dtype=np.float32)
    attn_bc = np.ascontiguousarray(np.broadcast_to(attn, (P, EMB + LSTM)))
    eye = np.eye(P, dtype=np.float32)
    in_maps = []
    for i in range(N_CORES):
        sl = slice(i * BLOC, (i + 1) * BLOC)
        in_maps.append(
            {
                # b-major so each device tile is one contiguous DRAM read
                "embedded": np.ascontiguousarray(
                    embedded[:, sl, :].transpose(1, 0, 2)
                ),
                "lstm_outputs": np.ascontiguousarray(
                    lstm_outputs[:, sl, :].transpose(1, 0, 2)
                ),
                "attn_bc": attn_bc,
                "ident": eye,
            }
        )
    return in_maps


def _run(embedded, lstm_outputs, attn, trace=False, **spmd_kwargs):
    nc = _get_nc()
    in_maps = _make_in_maps(embedded, lstm_outputs, attn)
    r = run_bass_kernel_spmd(
        nc, in_maps, core_ids=list(range(N_CORES)), trace=trace, **spmd_kwargs
    )
    out = np.concatenate([r.results[i]["out"] for i in range(N_CORES)], axis=0)
    return out, r


def kernel(embedded, lstm_outputs, attn, mask=None, **_ignored) -> np.ndarray:
    out, _ = _run(embedded, lstm_outputs, attn, trace=False)
    return out.astype(np.float32)


# revision 32
# speedup vs baseline: 1.0734x; 1.0734x over previous
"""Trainium2 Bass kernel for nn_AttentionLayer_10995116278518.

Computes softmax(einsum('sbe,e->bs', embedded, attn[:300])
              + einsum('sbf,f->bs', lstm_outputs, attn[300:]), axis=1)
(the reference's mask is computed-but-discarded, so it is unused here).

Sharding: data-parallel over batch. Each of the 8 cores handles 8 of the
64 batch rows; no cross-device communication.

Per-core device kernel layout:
  rows = (s, b) pairs; tiles put 128 consecutive s on partitions for a
  fixed b, features on the free axis. A fused VectorE tensor_tensor_reduce
  (multiply by the partition-broadcast attn vector, then add-reduce along
  the free axis) produces one dot product per partition. The per-row dots
  are collected as columns of L[128s, 4k*8b]; four PE transposes turn that
  into logits [8b, 512s] in PSUM, where softmax is a pure free-axis op.
"""

import os
import sys

import numpy as np

try:
    import concourse.bass as bass
except ImportError:  # stand-alone grading dir: the runtime lives here
    sys.path.insert(0, "/opt/trn_rl_repo")
    import concourse.bass as bass

import concourse.bacc as bacc
import concourse.tile as tile
from concourse import mybir
from concourse.bass_utils import run_bass_kernel_spmd

SEQ = 512
BATCH = 64
EMB = 300
LSTM = 4096
N_CORES = 8
BLOC = BATCH // N_CORES  # 8 batch rows per core
P = 128
NSB = SEQ // P  # 4 s-blocks of 128

F32 = mybir.dt.float32


def _build() -> bass.Bass:
    nc = bacc.Bacc()
    # host passes the embedded shard pre-tiled: [NSB, 128, BLOC, EMB]
    emb = nc.declare_dram_parameter(
        "embedded", [NSB, P, BLOC, EMB], F32, isOutput=False
    )
    # lstm shard transposed to b-major: [BLOC, SEQ, LSTM]
    lstm = nc.declare_dram_parameter(
        "lstm_outputs", [BLOC, SEQ, LSTM], F32, isOutput=False
    )
    attn_bc = nc.declare_dram_parameter("attn_bc", [P, EMB + LSTM], F32, isOutput=False)
    ident = nc.declare_dram_parameter("ident", [P, P], F32, isOutput=False)
    out = nc.declare_dram_parameter("out", [BLOC, SEQ], F32, isOutput=True)

    with tile.TileContext(nc) as tc:
        with (
            tc.tile_pool(name="singles", bufs=1) as singles,
            tc.tile_pool(name="lstm_tiles", bufs=3) as lstm_pool,
            tc.tile_pool(name="emb_tiles", bufs=4) as emb_pool,
            tc.tile_pool(name="psum", bufs=1, space="PSUM") as psum_pool,
        ):
            # setup loads on the sync ring; the gpsimd ring starts on lstm
            # tiles immediately
            sb_attn = singles.tile([P, EMB + LSTM], F32)
            nc.sync.dma_start(out=sb_attn, in_=attn_bc[:, :])
            attn_e = sb_attn[:, 0:EMB]
            attn_l = sb_attn[:, EMB : EMB + LSTM]
            sb_ident = singles.tile([P, P], F32)
            nc.sync.dma_start(out=sb_ident, in_=ident[:, :])

            # throwaway elementwise output of the ACT accumulating copy
            scratch = singles.tile([P, LSTM], F32)
            # per-row dot products: column k*BLOC+b holds rows (s=128k.., b)
            L = singles.tile([P, NSB * BLOC], F32)
            tmpl = singles.tile([P, NSB * BLOC], F32)  # lstm partials
            tmpe = singles.tile([P, NSB * BLOC], F32)  # embedded partials

            # all four embedded tiles up front (contiguous 1.2 MB each)
            emb_tiles = []
            for k in range(NSB):
                emb_t = emb_pool.tile([P, BLOC, EMB], F32)
                nc.sync.dma_start(out=emb_t, in_=emb[k])
                emb_tiles.append(emb_t)

            for kp in range(NSB // 2):
                for b in range(BLOC):
                    # 4 MB fully contiguous read of 256 consecutive s rows
                    lstm_t = lstm_pool.tile([P, 2, LSTM], F32)
                    dma_eng = nc.gpsimd if b % 2 == 0 else nc.sync
                    dma_eng.dma_start(
                        out=lstm_t,
                        in_=lstm[b, 2 * kp * P : (2 * kp + 2) * P, :].rearrange(
                            "(kk s) f -> s kk f", kk=2
                        ),
                    )
                    for kk in range(2):
                        k = 2 * kp + kk
                        col = k * BLOC + b
                        emb_t = emb_tiles[k]
                        # VectorE multiplies in place; ScalarE reduces
                        nc.vector.tensor_mul(
                            lstm_t[:, kk, :], lstm_t[:, kk, :], attn_l
                        )
                        nc.scalar.activation(
                            out=scratch,
                            in_=lstm_t[:, kk, :],
                            func=mybir.ActivationFunctionType.Copy,
                            accum_out=tmpl[:, col : col + 1],
                        )
                        nc.vector.tensor_mul(
                            emb_t[:, b, :], emb_t[:, b, :], attn_e
                        )
                        nc.scalar.activation(
                            out=scratch[:, 0:EMB],
                            in_=emb_t[:, b, :],
                            func=mybir.ActivationFunctionType.Copy,
                            accum_out=tmpe[:, col : col + 1],
                        )

            nc.vector.tensor_add(L, tmpl, tmpe)

            # logits [8b, 512s] in PSUM via four PE transposes of [128, 8]
            logits = psum_pool.tile([BLOC, SEQ], F32)
            for k in range(NSB):
                nc.tensor.transpose(
                    out=logits[:, k * P : (k + 1) * P],
                    in_=L[:, k * BLOC : (k + 1) * BLOC],
                    identity=sb_ident,
                )

            # softmax along s (free axis)
            m = singles.tile([BLOC, 1], F32)
            nm = singles.tile([BLOC, 1], F32)
            ssum = singles.tile([BLOC, 1], F32)
            rec = singles.tile([BLOC, 1], F32)
            expt = singles.tile([BLOC, SEQ], F32)
            res = singles.tile([BLOC, SEQ], F32)
            nc.vector.reduce_max(out=m, in_=logits, axis=mybir.AxisListType.X)
            nc.vector.tensor_scalar_mul(nm, m, -1.0)
            nc.scalar.activation(
                out=expt,
                in_=logits,
                func=mybir.ActivationFunctionType.Exp,
                bias=nm,
                scale=1.0,
                accum_out=ssum,
            )
            nc.vector.reciprocal(rec, ssum)
            nc.vector.tensor_scalar_mul(res, expt, rec)
            nc.sync.dma_start(out=out[:, :], in_=res)

    nc.compile()
    return nc


_NC_CACHE = None


def _get_nc() -> bass.Bass:
    global _NC_CACHE
    if _NC_CACHE is None:
        _NC_CACHE = _build()
    return _NC_CACHE


def _make_in_maps(embedded, lstm_outputs, attn):
    embedded = np.asarray(embedded, dtype=np.float32)
    lstm_outputs = np.asarray(lstm_outputs, dtype=np.float32)
    attn = np.asarray(attn, dtype=np.float32)
    attn_bc = np.ascontiguousarray(np.broadcast_to(attn, (P, EMB + LSTM)))
    eye = np.eye(P, dtype=np.float32)
    in_maps = []
    for i in range(N_CORES):
        sl = slice(i * BLOC, (i + 1) * BLOC)
        in_maps.append(
            {
                # pre-tiled / b-major so each device tile is one
                # contiguous DRAM read
                "embedded": np.ascontiguousarray(
                    embedded[:, sl, :].reshape(NSB, P, BLOC, EMB)
                ),
                "lstm_outputs": np.ascontiguousarray(
                    lstm_outputs[:, sl, :].transpose(1, 0, 2)
                ),
                "attn_bc": attn_bc,
                "ident": eye,
            }
        )
    return in_maps


def _run(embedded, lstm_outputs, attn, trace=False, **spmd_kwargs):
    nc = _get_nc()
    in_maps = _make_in_maps(embedded, lstm_outputs, attn)
    r = run_bass_kernel_spmd(
        nc, in_maps, core_ids=list(range(N_CORES)), trace=trace, **spmd_kwargs
    )
    out = np.concatenate([r.results[i]["out"] for i in range(N_CORES)], axis=0)
    return out, r


def kernel(embedded, lstm_outputs, attn, mask=None, **_ignored) -> np.ndarray:
    out, _ = _run(embedded, lstm_outputs, attn, trace=False)
    return out.astype(np.float32)
